# revision 32
# baseline (speedup 1.0000x reference)
"""Trainium2 Bass kernel for per-aspect 2-layer MLP (embedding-lookup MLP).

Reference computation (B=1024, D=768, H=256, A=20, T=2):
    W1 = W1_embs[aspect_ids].reshape(B, D, H)
    out1 = relu(X @batched W1 + b1_embs[aspect_ids])
    logits = out1 @batched W2_embs[aspect_ids].reshape(B, H, T) + b2_embs[aspect_ids]

Strategy: only A=20 distinct aspects exist, so group samples by aspect on
the host and turn the per-sample batched matvec into one dense matmul per
aspect.  Shard the aspect-groups across the 8 NeuronCores so the big
weight table is read from HBM exactly once chip-wide (~16MB total)
instead of once per sample (~800MB):

  - A // 8 = 2 "full" slot classes: core c serves aspects rank[c] and
    rank[8+c] (groups assigned by size rank so slot j has the same padded
    size S_j on every core -> SPMD-uniform program).
  - the A % 8 = 4 leftover aspects are split along the HIDDEN dim: two
    cores each take 128 of the 256 hidden units (relu is elementwise, so
    each half is self-contained through layer 1; layer 2 contributions
    are partial sums the host adds during the gather).  This balances W1
    bytes perfectly: 20 x 786KB / 8 = 1.97MB per core.

Device program per slot (S = padded group size, chunks of <=128 samples):
  - two ~0.5MB DMAs (k<3 half on the SP HWDGE ring, k>=3 half on the ACT
    ring; per-ring FIFO keeps arrivals in compute order, two rings keep
    two transfers in flight) load the host-packed [128, F] half-slabs:
    W1 as [128,h_j] rhs chunks + X^T as [128,S] lhsT (stationary) chunks.
  - layer 1 on PE: psum[S,h_j] accumulates 6 matmuls in float32r
    (single-pass fp32 mode, full 1 cycle/row stream rate at N>=256) plus
    a 7th K=1 matmul (ones[1,S] x b1[1,h_j]) adding the bias.
  - w2 columns are replicated across partitions by a K=1 PE matmul
    (ones[128] x w2row) and copied to SBUF.
  - layer 2 on DVE: one fused scalar_tensor_tensor per logit column
    computes (psum max 0) * w2col with accum_out = per-sample sum (relu
    fused in, layer-1 psum read directly), then a tiny tensor_add adds b2
    (host packs b2 = 0 for the second half of a split aspect).
  - logits live as [S,2] column pairs of a [128, 2*n_units] tile; one
    final DMA (on the otherwise-idle ACT ring) stores it.

BIR post-passes work around toolchain limits and shave startup: splitting
>1-sync-wait instructions (this walrus rejects them), and hoisting the
wait-free input DMA triggers above the program entry barrier so HBM
transfers run while engines initialize (~6us saved).

float32r is the TRN2 single-pass fp32 matmul mode: ~1.5e-4 relative
error vs ~1e-7 for the 2-pass fp32 mode, ~2.7x faster.  VARIANT="fp32"
gives bit-accurate 2-pass matmuls at ~+6us; "bf16" halves DMA bytes but
measured slower here (cold-PE-bound) with ~3.5e-3 error.
"""

import numpy as np

N_CORES = 8
PART = 128
VARIANT = "fp32r"  # "fp32r" | "fp32" | "bf16"

_cache: dict = {}


# ───────────────────────── BIR post-passes ─────────────────────────

def _split_excess_waits(nc):
    """This walrus build rejects >1 sync-wait on one instruction (seen on
    the TileContext tail Drain).  Hoist excess sem waits onto preceding
    NoOps on the same engine — semantically identical (program order)."""
    import concourse.mybir as mybir
    import bass_rust

    n_new = 0
    for f in nc.m.functions:
        for bb in f.blocks:
            insts = bb.instructions
            out = []
            changed = False
            for inst in insts:
                si = inst.sync_info
                if si is not None and si.on_wait and len(si.on_wait) > 1:
                    waits = list(si.on_wait)
                    keep = [w for w in waits if w.wait_reg is not None]
                    movable = [w for w in waits if w.wait_reg is None]
                    while len(keep) < 1 and movable:
                        keep.append(movable.pop())
                    for w in movable:
                        nop = mybir.InstNoOp(
                            name=f"waitsplit_{n_new}", engine=inst.engine,
                            sync_info=bass_rust.SyncInfo(on_wait=[w], on_update=[]))
                        n_new += 1
                        out.append(nop)
                    inst.sync_info = bass_rust.SyncInfo(
                        on_wait=keep, on_update=list(si.on_update))
                    changed = True
                out.append(inst)
            if changed:
                bb.instructions = out
    return n_new


def _hoist_initial_dmas(nc):
    """Move wait-free input-DMA triggers from the tile body to before the
    program's entry barrier on their issuing engine, so HBM transfers start
    while the engines are still initializing (saves ~6us of startup)."""
    import concourse.mybir as mybir

    f = nc.m.functions[0]
    bbs = list(f.blocks)
    if len(bbs) < 2:
        return 0
    main_bb, body_bb = bbs[0], bbs[1]

    body = body_bb.instructions
    hoisted = {}  # engine -> list[inst]
    remaining = []
    blocked = set()  # engines whose stream hit a non-hoistable inst
    for inst in body:
        eng = inst.engine
        si = inst.sync_info
        is_dma = isinstance(inst, mybir.InstDMACopy)
        waitfree = si is None or not si.on_wait
        if is_dma and waitfree and eng not in blocked:
            hoisted.setdefault(eng, []).append(inst)
        else:
            if eng != mybir.EngineType.Unassigned:
                blocked.add(eng)
            remaining.append(inst)
    if not hoisted:
        return 0

    main = main_bb.instructions
    out = []
    placed = set()
    # insert right before the engine's entry Drain
    for inst in main:
        if (isinstance(inst, mybir.InstDrain) and inst.engine in hoisted
                and inst.engine not in placed):
            out.extend(hoisted[inst.engine])
            placed.add(inst.engine)
        out.append(inst)
    for eng, insts in hoisted.items():
        if eng not in placed:
            out.extend(insts)
    main_bb.instructions = out
    body_bb.instructions = remaining
    return sum(len(v) for v in hoisted.values())


# ───────────────────────── shared layout ─────────────────────────

T_OUT = 2


def _classes(A, H, mh):
    """Slot classes: (n_full full-H slots, split: bool).  The leftover
    A % 8 aspects are h-split across two cores when that covers <= all
    cores and H has an even number of 128-chunks."""
    n_full = A // N_CORES
    rem = A % N_CORES
    hs = [H] * n_full
    split = False
    if rem:
        if mh % 2 == 0 and 2 * rem <= N_CORES:
            hs.append(H // 2)
            split = True
        else:
            hs.append(H)
    return hs, split


def _layout(s_sizes, hs, d):
    """Per slot TWO half-slabs (k-groups), each [128, 3*h_j + 3*S_j]."""
    kd = d // PART
    kh = kd // 2
    offs, fs = [], []
    for s, h in zip(s_sizes, hs):
        o_xt = kh * h
        f = o_xt + kh * s
        f += (-f) % 8
        offs.append(o_xt)
        fs.append(f)
    return offs, fs


def _smr_layout(hs):
    """SMALLR row: per-slot [b1[h_j] | w2col0,b2_0 [h_j+2] | w2col1,b2_1
    [h_j+2]], then ones[PART]."""
    offs = []
    off = 0
    for h in hs:
        offs.append(off)
        per = h + T_OUT * (h + 2)
        per += (-per) % 8
        off += per
    return offs, off, off + PART  # slot offsets, ones offset, total


def _units(s_sizes):
    """(slot, s0, sc) chunks of <=128 samples."""
    us = []
    for j, s in enumerate(s_sizes):
        for s0 in range(0, s, PART):
            us.append((j, s0, min(PART, s - s0)))
    return us


# ───────────────────────── device program ─────────────────────────

def _build_nc(s_sizes, hs, d, variant):
    import concourse.bass as bass
    import concourse.mybir as mybir
    from concourse.tile import TileContext

    fp32 = mybir.dt.float32
    mmdt = {"fp32r": mybir.dt.float32r,
            "bf16": mybir.dt.bfloat16,
            "fp32": fp32}[variant]
    kd = d // PART
    kh = kd // 2
    T = T_OUT
    offs, fs = _layout(s_sizes, hs, d)
    ftot = 2 * sum(fs)
    units = _units(s_sizes)
    n_slots = len(s_sizes)
    smr_offs, smr_ones, smr_tot = _smr_layout(hs)
    hmax = max(hs)

    nc = bass.Bass()
    IN = nc.dram_tensor("IN", [PART, ftot], mmdt, kind="ExternalInput")
    SMALLR = nc.dram_tensor("SMALLR", [1, smr_tot], mmdt, kind="ExternalInput")
    OUT = nc.dram_tensor("OUT", [PART, T * len(units)], fp32,
                         kind="ExternalOutput")

    with TileContext(nc) as tc:
        with tc.tile_pool(name="inp", bufs=2 * n_slots) as inp_pool, \
             tc.tile_pool(name="smallp", bufs=1) as small_pool, \
             tc.tile_pool(name="w2sbp", bufs=T * n_slots) as w2sb_pool, \
             tc.tile_pool(name="scrp", bufs=2) as scr_pool, \
             tc.tile_pool(name="outp", bufs=1) as out_pool, \
             tc.tile_pool(name="ps1", bufs=2, space="PSUM") as ps1_pool, \
             tc.tile_pool(name="psw", bufs=T * n_slots, space="PSUM") as psw_pool:

            out_sb = out_pool.tile([PART, T * len(units)], fp32)
            small_t = small_pool.tile([1, smr_tot], mmdt)

            # prefetch all input half-slabs, k<3 halves on the sync ring and
            # k>=3 halves on the scalar ring (per-ring FIFO => data lands in
            # compute order; two rings keep two transfers in flight).
            in_ts = []
            in_off = 0
            for j in range(n_slots):
                pair = []
                for g in range(2):
                    in_t = inp_pool.tile([PART, fs[j]], mmdt, tag="in_t")
                    eng = nc.sync if g == 0 else nc.scalar
                    eng.dma_start(
                        out=in_t[:], in_=IN[:, in_off:in_off + fs[j]])
                    pair.append(in_t)
                    in_off += fs[j]
                    if j == 0 and g == 1:
                        nc.scalar.dma_start(out=small_t[:], in_=SMALLR[:])
                in_ts.append(pair)

            ones_full = small_t[0:1, smr_ones:smr_ones + PART]

            # replicate each slot's w2 column (+b2) across all partitions on
            # the PE: psum[p, f] = ones[p] * w2row[f]; copy to SBUF for DVE.
            w2sb = []
            for j in range(n_slots):
                h = hs[j]
                for t in range(T):
                    wp = psw_pool.tile([PART, h + 2], fp32, tag="w2ps")
                    src = small_t[0:1, smr_offs[j] + h + t * (h + 2):
                                  smr_offs[j] + h + (t + 1) * (h + 2)]
                    nc.tensor.matmul(wp[:], ones_full, src,
                                     start=True, stop=True)
                    wsb = w2sb_pool.tile([PART, hmax + 2], fp32, tag="w2sb")
                    nc.vector.tensor_copy(out=wsb[:, 0:h + 2], in_=wp[:])
                    w2sb.append(wsb)

            for ui, (j, s0, sc) in enumerate(units):
                s = s_sizes[j]
                h = hs[j]
                o_xt = offs[j]
                b1row = small_t[0:1, smr_offs[j]:smr_offs[j] + h]
                ones = small_t[0:1, smr_ones + 0:smr_ones + sc]

                ps = ps1_pool.tile([sc, h], fp32, tag="ps")
                for k in range(kd):
                    in_t = in_ts[j][k // kh]
                    kk = k % kh
                    nc.tensor.matmul(
                        ps[:],
                        in_t[:, o_xt + kk * s + s0:o_xt + kk * s + s0 + sc],
                        in_t[:, kk * h:(kk + 1) * h],
                        start=(k == 0), stop=False)
                nc.tensor.matmul(
                    ps[:], ones, b1row, start=False, stop=True)

                for t in range(T):
                    wsb = w2sb[j * T + t]
                    scr = scr_pool.tile([PART, hmax], fp32, tag=f"scr{t}")
                    acc = scr_pool.tile([PART, 1], fp32, tag=f"acc{t}")
                    # logits col = sum_h(relu(psum) * w2[:,t]) in one DVE op
                    nc.vector.scalar_tensor_tensor(
                        out=scr[:sc, 0:h], in0=ps[:], scalar=0.0,
                        in1=wsb[:sc, 0:h],
                        op0=mybir.AluOpType.max,
                        op1=mybir.AluOpType.mult,
                        accum_out=acc[:sc, 0:1])
                    nc.vector.tensor_add(
                        out=out_sb[:sc, T * ui + t:T * ui + t + 1],
                        in0=acc[:sc, 0:1],
                        in1=wsb[:sc, h:h + 1])
            nc.scalar.dma_start(out=OUT[:], in_=out_sb[:])

    _split_excess_waits(nc)
    _hoist_initial_dmas(nc)
    return nc


# ───────────────────────── host side ─────────────────────────

def _install_ntff_hook():
    import sys, types
    if "antenv.axon_hooks" in sys.modules:
        return
    import antenv
    from trn_agent_boot.trn_boot import _ntff_profile_via_ctypes
    mod = types.ModuleType("antenv.axon_hooks")
    hook = _ntff_profile_via_ctypes('/opt/axon/libaxon_pjrt.so')
    mod.get_axon_ntff_profile_hook = lambda: hook
    mod.set_axon_ntff_profile_hook = lambda h: None
    sys.modules["antenv.axon_hooks"] = mod
    antenv.axon_hooks = mod


def _slot_assign(c, j, hs, split, rank, A, H):
    """Aspect + hidden-range served by (core c, slot j).
    Returns (aspect or -1, h_off, h_len)."""
    n_full = sum(1 for h in hs[:j] if True)  # j == index
    h = hs[j]
    if h == H or not split or j < len(hs) - 1:
        r = j * N_CORES + c
        return (int(rank[r]) if r < A else -1), 0, h
    # split class: two cores per aspect, one H-half each
    base = j * N_CORES
    ai = c // 2
    r = base + ai
    if r < A:
        return int(rank[r]), (c % 2) * h, h
    return -1, 0, h


def _run(X, aspect_ids, W1_embs, b1_embs, W2_embs, b2_embs, trace=False):
    B, D = X.shape
    A, H = b1_embs.shape
    T = b2_embs.shape[1]
    assert D % PART == 0 and H % PART == 0 and T == T_OUT
    kd, mh = D // PART, H // PART
    kh = kd // 2

    X = np.ascontiguousarray(X, dtype=np.float32)
    W1_embs = np.ascontiguousarray(W1_embs, dtype=np.float32)
    b1_embs = np.ascontiguousarray(b1_embs, dtype=np.float32)
    W2_embs = np.ascontiguousarray(W2_embs, dtype=np.float32)
    b2_embs = np.ascontiguousarray(b2_embs, dtype=np.float32)
    ids = np.asarray(aspect_ids).astype(np.int64)

    order = np.argsort(ids, kind="stable")
    counts = np.bincount(ids, minlength=A)
    starts = np.concatenate([[0], np.cumsum(counts)])
    rank = np.argsort(-counts, kind="stable")

    hs, split = _classes(A, H, mh)
    n_slots = len(hs)
    s_sizes = []
    for j in range(n_slots):
        if split and j == n_slots - 1:
            cls = rank[j * N_CORES:A]
        else:
            cls = rank[j * N_CORES:(j + 1) * N_CORES]
        smax = max(1, int(counts[cls].max()) if len(cls) else 1)
        smax += (-smax) % 8
        s_sizes.append(smax)

    offs, fs = _layout(s_sizes, hs, D)
    ftot = 2 * sum(fs)
    units = _units(s_sizes)
    smr_offs, smr_ones, smr_tot = _smr_layout(hs)

    key = (tuple(s_sizes), tuple(hs), D, VARIANT)
    if key not in _cache:
        _cache[key] = _build_nc(s_sizes, hs, D, VARIANT)
    nc = _cache[key]

    in_maps = []
    scatter = []  # (core, unit_idx, idx_global_rows)
    for c in range(N_CORES):
        buf = np.zeros((PART, ftot), dtype=np.float32)
        smr = np.zeros((1, smr_tot), dtype=np.float32)
        smr[0, smr_ones:smr_ones + PART] = 1.0
        in_off = 0
        for j, s in enumerate(s_sizes):
            h = hs[j]
            a, h_off, _ = _slot_assign(c, j, hs, split, rank, A, H)
            if a >= 0:
                n_a = int(counts[a])
                idx = order[starts[a]:starts[a] + n_a]
                w1p = (W1_embs[a].reshape(kd, PART, H)
                       [:, :, h_off:h_off + h]
                       .transpose(1, 0, 2).reshape(PART, kd * h))
                if n_a > 0:
                    pidx = np.concatenate([idx, np.repeat(idx[:1], s - n_a)])
                    xtp = (X[pidx].T.reshape(kd, PART, s)
                           .transpose(1, 0, 2).reshape(PART, kd * s))
                else:
                    xtp = np.zeros((PART, kd * s), dtype=np.float32)
                for g in range(2):
                    base = in_off + g * fs[j]
                    buf[:, base:base + kh * h] = (
                        w1p[:, g * kh * h:(g + 1) * kh * h])
                    buf[:, base + offs[j]:base + offs[j] + kh * s] = (
                        xtp[:, g * kh * s:(g + 1) * kh * s])
                smr[0, smr_offs[j]:smr_offs[j] + h] = (
                    b1_embs[a][h_off:h_off + h])
                w2c = W2_embs[a].reshape(H, T)
                for t in range(T):
                    base = smr_offs[j] + h + t * (h + 2)
                    smr[0, base:base + h] = w2c[h_off:h_off + h, t]
                    smr[0, base + h] = (
                        b2_embs[a][t] if h_off == 0 else 0.0)
                for ui, (jj, s0, sc) in enumerate(units):
                    if jj == j and s0 < n_a:
                        scatter.append((c, ui, idx[s0:s0 + sc]))
            in_off += 2 * fs[j]
        if VARIANT == "bf16":
            import ml_dtypes
            buf = buf.astype(ml_dtypes.bfloat16)
            smr = smr.astype(ml_dtypes.bfloat16)
        in_maps.append({"IN": buf, "SMALLR": smr})

    if trace:
        _install_ntff_hook()
    from concourse import bass_utils
    bass_utils.upload_artifacts = lambda tmpdir: str(tmpdir)
    res = bass_utils.run_bass_kernel_spmd(
        nc, in_maps, list(range(N_CORES)), trace=trace)

    # gather: split-aspect halves produce partial logits -> accumulate
    out = np.zeros((B, T), dtype=np.float32)
    for c, ui, idx in scatter:
        out[idx] += res.results[c]["OUT"][:len(idx), T * ui:T * ui + T]
    return out, res


def kernel(**inputs):
    out, _ = _run(**inputs)
    return out


# revision 33
# speedup vs baseline: 1.1103x; 1.1103x over previous
"""Trainium2 Bass kernel for per-aspect 2-layer MLP (embedding-lookup MLP).

Reference computation (B=1024, D=768, H=256, A=20, T=2):
    W1 = W1_embs[aspect_ids].reshape(B, D, H)
    out1 = relu(X @batched W1 + b1_embs[aspect_ids])
    logits = out1 @batched W2_embs[aspect_ids].reshape(B, H, T) + b2_embs[aspect_ids]

Strategy: only A=20 distinct aspects exist, so group samples by aspect on
the host and turn the per-sample batched matvec into one dense matmul per
aspect.  Shard the aspect-groups across the 8 NeuronCores so the big
weight table is read from HBM exactly once chip-wide (~16MB total)
instead of once per sample (~800MB):

  - A // 8 = 2 "full" slot classes: core c serves aspects rank[c] and
    rank[8+c] (groups assigned by size rank so slot j has the same padded
    size S_j on every core -> SPMD-uniform program).
  - the A % 8 = 4 leftover aspects are split along the HIDDEN dim: two
    cores each take 128 of the 256 hidden units (relu is elementwise, so
    each half is self-contained through layer 1; layer 2 contributions
    are partial sums the host adds during the gather).  This balances W1
    bytes perfectly: 20 x 786KB / 8 = 1.97MB per core.

Device program per slot (S = padded group size, chunks of <=128 samples):
  - two ~0.5MB DMAs (k<3 half on the SP HWDGE ring, k>=3 half on the ACT
    ring; per-ring FIFO keeps arrivals in compute order, two rings keep
    two transfers in flight) load the host-packed [128, F] half-slabs:
    W1 as [128,h_j] rhs chunks + X^T as [128,S] lhsT (stationary) chunks.
  - layer 1 on PE: psum[S,h_j] accumulates 6 matmuls in float32r
    (single-pass fp32 mode, full 1 cycle/row stream rate at N>=256) plus
    a 7th K=1 matmul (ones[1,S] x b1[1,h_j]) adding the bias.
  - w2 columns are replicated across partitions by a K=1 PE matmul
    (ones[128] x w2row) and copied to SBUF.
  - layer 2 on DVE: one fused scalar_tensor_tensor per logit column
    computes (psum max 0) * w2col with accum_out = per-sample sum (relu
    fused in, layer-1 psum read directly), then a tiny tensor_add adds b2
    (host packs b2 = 0 for the second half of a split aspect).
  - logits live as [S,2] column pairs of a [128, 2*n_units] tile; one
    final DMA (on the otherwise-idle ACT ring) stores it.

BIR post-passes work around toolchain limits and shave startup: splitting
>1-sync-wait instructions (this walrus rejects them), and hoisting the
wait-free input DMA triggers above the program entry barrier so HBM
transfers run while engines initialize (~6us saved).

float32r is the TRN2 single-pass fp32 matmul mode: ~1.5e-4 relative
error vs ~1e-7 for the 2-pass fp32 mode, ~2.7x faster.  VARIANT="fp32"
gives bit-accurate 2-pass matmuls at ~+6us; "bf16" halves DMA bytes but
measured slower here (cold-PE-bound) with ~3.5e-3 error.
"""

import numpy as np

N_CORES = 8
PART = 128
VARIANT = "fp32r"  # "fp32r" | "fp32" | "bf16"

_cache: dict = {}


# ───────────────────────── BIR post-passes ─────────────────────────

def _split_excess_waits(nc):
    """This walrus build rejects >1 sync-wait on one instruction (seen on
    the TileContext tail Drain).  Hoist excess sem waits onto preceding
    NoOps on the same engine — semantically identical (program order)."""
    import concourse.mybir as mybir
    import bass_rust

    n_new = 0
    for f in nc.m.functions:
        for bb in f.blocks:
            insts = bb.instructions
            out = []
            changed = False
            for inst in insts:
                si = inst.sync_info
                if si is not None and si.on_wait and len(si.on_wait) > 1:
                    waits = list(si.on_wait)
                    keep = [w for w in waits if w.wait_reg is not None]
                    movable = [w for w in waits if w.wait_reg is None]
                    while len(keep) < 1 and movable:
                        keep.append(movable.pop())
                    for w in movable:
                        nop = mybir.InstNoOp(
                            name=f"waitsplit_{n_new}", engine=inst.engine,
                            sync_info=bass_rust.SyncInfo(on_wait=[w], on_update=[]))
                        n_new += 1
                        out.append(nop)
                    inst.sync_info = bass_rust.SyncInfo(
                        on_wait=keep, on_update=list(si.on_update))
                    changed = True
                out.append(inst)
            if changed:
                bb.instructions = out
    return n_new


def _hoist_initial_dmas(nc):
    """Move wait-free input-DMA triggers from the tile body to before the
    program's entry barrier on their issuing engine, so HBM transfers start
    while the engines are still initializing (saves ~6us of startup)."""
    import concourse.mybir as mybir

    f = nc.m.functions[0]
    bbs = list(f.blocks)
    if len(bbs) < 2:
        return 0
    main_bb, body_bb = bbs[0], bbs[1]

    body = body_bb.instructions
    hoisted = {}  # engine -> list[inst]
    remaining = []
    blocked = set()  # engines whose stream hit a non-hoistable inst
    for inst in body:
        eng = inst.engine
        si = inst.sync_info
        is_dma = isinstance(inst, mybir.InstDMACopy)
        waitfree = si is None or not si.on_wait
        if is_dma and waitfree and eng not in blocked:
            hoisted.setdefault(eng, []).append(inst)
        else:
            if eng != mybir.EngineType.Unassigned:
                blocked.add(eng)
            remaining.append(inst)
    if not hoisted:
        return 0

    main = main_bb.instructions
    out = []
    placed = set()
    # insert right before the engine's entry Drain
    for inst in main:
        if (isinstance(inst, mybir.InstDrain) and inst.engine in hoisted
                and inst.engine not in placed):
            out.extend(hoisted[inst.engine])
            placed.add(inst.engine)
        out.append(inst)
    for eng, insts in hoisted.items():
        if eng not in placed:
            out.extend(insts)
    main_bb.instructions = out
    body_bb.instructions = remaining
    return sum(len(v) for v in hoisted.values())


# ───────────────────────── shared layout ─────────────────────────

T_OUT = 2


def _classes(A, H, mh):
    """Slot classes: (n_full full-H slots, split: bool).  The leftover
    A % 8 aspects are h-split across two cores when that covers <= all
    cores and H has an even number of 128-chunks."""
    n_full = A // N_CORES
    rem = A % N_CORES
    hs = [H] * n_full
    split = False
    if rem:
        if mh % 2 == 0 and 2 * rem <= N_CORES:
            hs.append(H // 2)
            split = True
        else:
            hs.append(H)
    return hs, split


def _layout(s_sizes, hs, d):
    """Per slot TWO half-slabs (k-groups), each [128, 3*h_j + 3*S_j]."""
    kd = d // PART
    kh = kd // 2
    offs, fs = [], []
    for s, h in zip(s_sizes, hs):
        o_xt = kh * h
        f = o_xt + kh * s
        f += (-f) % 8
        offs.append(o_xt)
        fs.append(f)
    return offs, fs


def _smr_layout(hs):
    """SMALLR row: per-slot [b1[h_j] | w2col0,b2_0 [h_j+2] | w2col1,b2_1
    [h_j+2]], then ones[PART]."""
    offs = []
    off = 0
    for h in hs:
        offs.append(off)
        per = h + T_OUT * (h + 2)
        per += (-per) % 8
        off += per
    return offs, off, off + PART  # slot offsets, ones offset, total


def _units(s_sizes):
    """(slot, s0, sc) chunks of <=128 samples."""
    us = []
    for j, s in enumerate(s_sizes):
        for s0 in range(0, s, PART):
            us.append((j, s0, min(PART, s - s0)))
    return us


# ───────────────────────── device program ─────────────────────────

def _build_nc(s_sizes, hs, d, variant):
    import concourse.bass as bass
    import concourse.mybir as mybir
    from concourse.tile import TileContext

    fp32 = mybir.dt.float32
    mmdt = {"fp32r": mybir.dt.float32r,
            "bf16": mybir.dt.bfloat16,
            "fp32": fp32}[variant]
    kd = d // PART
    kh = kd // 2
    T = T_OUT
    offs, fs = _layout(s_sizes, hs, d)
    ftot = 2 * sum(fs)
    units = _units(s_sizes)
    n_slots = len(s_sizes)
    smr_offs, smr_ones, smr_tot = _smr_layout(hs)
    hmax = max(hs)

    nc = bass.Bass()
    IN = nc.dram_tensor("IN", [PART, ftot], mmdt, kind="ExternalInput")
    SMALLR = nc.dram_tensor("SMALLR", [1, smr_tot], mmdt, kind="ExternalInput")
    OUT = nc.dram_tensor("OUT", [PART, T * len(units)], fp32,
                         kind="ExternalOutput")

    with TileContext(nc) as tc:
        with tc.tile_pool(name="inp", bufs=2 * n_slots) as inp_pool, \
             tc.tile_pool(name="smallp", bufs=1) as small_pool, \
             tc.tile_pool(name="w2sbp", bufs=T * n_slots) as w2sb_pool, \
             tc.tile_pool(name="scrp", bufs=2) as scr_pool, \
             tc.tile_pool(name="outp", bufs=1) as out_pool, \
             tc.tile_pool(name="ps1", bufs=2, space="PSUM") as ps1_pool, \
             tc.tile_pool(name="psw", bufs=T * n_slots, space="PSUM") as psw_pool:

            out_sb = out_pool.tile([PART, T * len(units)], fp32)
            small_t = small_pool.tile([1, smr_tot], mmdt)

            # prefetch all input half-slabs, k<3 halves on the sync ring and
            # k>=3 halves on the scalar ring (per-ring FIFO => data lands in
            # compute order; two rings keep two transfers in flight).
            in_ts = []
            in_off = 0
            for j in range(n_slots):
                pair = []
                for g in range(2):
                    in_t = inp_pool.tile([PART, fs[j]], mmdt, tag="in_t")
                    eng = nc.sync if g == 0 else nc.scalar
                    eng.dma_start(
                        out=in_t[:], in_=IN[:, in_off:in_off + fs[j]])
                    pair.append(in_t)
                    in_off += fs[j]
                    if j == 0 and g == 1:
                        nc.scalar.dma_start(out=small_t[:], in_=SMALLR[:])
                in_ts.append(pair)

            ones_full = small_t[0:1, smr_ones:smr_ones + PART]

            # replicate each slot's w2 column (+b2) across all partitions on
            # the PE: psum[p, f] = ones[p] * w2row[f]; copy to SBUF for DVE.
            w2sb = []
            for j in range(n_slots):
                h = hs[j]
                for t in range(T):
                    wp = psw_pool.tile([PART, h + 2], fp32, tag="w2ps")
                    src = small_t[0:1, smr_offs[j] + h + t * (h + 2):
                                  smr_offs[j] + h + (t + 1) * (h + 2)]
                    nc.tensor.matmul(wp[:], ones_full, src,
                                     start=True, stop=True)
                    wsb = w2sb_pool.tile([PART, hmax + 2], fp32, tag="w2sb")
                    nc.vector.tensor_copy(out=wsb[:, 0:h + 2], in_=wp[:])
                    w2sb.append(wsb)

            for ui, (j, s0, sc) in enumerate(units):
                s = s_sizes[j]
                h = hs[j]
                o_xt = offs[j]
                b1row = small_t[0:1, smr_offs[j]:smr_offs[j] + h]
                ones = small_t[0:1, smr_ones + 0:smr_ones + sc]

                ps = ps1_pool.tile([sc, h], fp32, tag="ps")
                for k in range(kd):
                    in_t = in_ts[j][k // kh]
                    kk = k % kh
                    nc.tensor.matmul(
                        ps[:],
                        in_t[:, o_xt + kk * s + s0:o_xt + kk * s + s0 + sc],
                        in_t[:, kk * h:(kk + 1) * h],
                        start=(k == 0), stop=False)
                nc.tensor.matmul(
                    ps[:], ones, b1row, start=False, stop=True)

                for t in range(T):
                    wsb = w2sb[j * T + t]
                    scr = scr_pool.tile([PART, hmax], fp32, tag=f"scr{t}")
                    acc = scr_pool.tile([PART, 1], fp32, tag=f"acc{t}")
                    # logits col = sum_h(relu(psum) * w2[:,t]) in one DVE op
                    nc.vector.scalar_tensor_tensor(
                        out=scr[:sc, 0:h], in0=ps[:], scalar=0.0,
                        in1=wsb[:sc, 0:h],
                        op0=mybir.AluOpType.max,
                        op1=mybir.AluOpType.mult,
                        accum_out=acc[:sc, 0:1])
                    nc.vector.tensor_add(
                        out=out_sb[:sc, T * ui + t:T * ui + t + 1],
                        in0=acc[:sc, 0:1],
                        in1=wsb[:sc, h:h + 1])
            nc.scalar.dma_start(out=OUT[:], in_=out_sb[:])

    _split_excess_waits(nc)
    _hoist_initial_dmas(nc)
    return nc


# ───────────────────────── host side ─────────────────────────

def _install_ntff_hook():
    import sys, types
    if "antenv.axon_hooks" in sys.modules:
        return
    import antenv
    from trn_agent_boot.trn_boot import _ntff_profile_via_ctypes
    mod = types.ModuleType("antenv.axon_hooks")
    hook = _ntff_profile_via_ctypes('/opt/axon/libaxon_pjrt.so')
    mod.get_axon_ntff_profile_hook = lambda: hook
    mod.set_axon_ntff_profile_hook = lambda h: None
    sys.modules["antenv.axon_hooks"] = mod
    antenv.axon_hooks = mod


def _slot_assign(c, j, hs, split, rank, A, H):
    """Aspect + hidden-range served by (core c, slot j).
    Returns (aspect or -1, h_off, h_len)."""
    h = hs[j]
    if h == H or not split or j < len(hs) - 1:
        r = j * N_CORES + c
        return (int(rank[r]) if r < A else -1), 0, h
    # split class: two cores per aspect, one H-half each
    base = j * N_CORES
    ai = c // 2
    r = base + ai
    if r < A:
        return int(rank[r]), (c % 2) * h, h
    return -1, 0, h


def _run(X, aspect_ids, W1_embs, b1_embs, W2_embs, b2_embs, trace=False):
    B, D = X.shape
    A, H = b1_embs.shape
    T = b2_embs.shape[1]
    assert D % PART == 0 and H % PART == 0 and T == T_OUT
    kd, mh = D // PART, H // PART
    kh = kd // 2

    X = np.ascontiguousarray(X, dtype=np.float32)
    W1_embs = np.ascontiguousarray(W1_embs, dtype=np.float32)
    b1_embs = np.ascontiguousarray(b1_embs, dtype=np.float32)
    W2_embs = np.ascontiguousarray(W2_embs, dtype=np.float32)
    b2_embs = np.ascontiguousarray(b2_embs, dtype=np.float32)
    ids = np.asarray(aspect_ids).astype(np.int64)

    order = np.argsort(ids, kind="stable")
    counts = np.bincount(ids, minlength=A)
    starts = np.concatenate([[0], np.cumsum(counts)])
    rank = np.argsort(-counts, kind="stable")

    hs, split = _classes(A, H, mh)
    n_slots = len(hs)
    s_sizes = []
    for j in range(n_slots):
        if split and j == n_slots - 1:
            cls = rank[j * N_CORES:A]
        else:
            cls = rank[j * N_CORES:(j + 1) * N_CORES]
        smax = max(1, int(counts[cls].max()) if len(cls) else 1)
        smax += (-smax) % 8
        s_sizes.append(smax)

    offs, fs = _layout(s_sizes, hs, D)
    ftot = 2 * sum(fs)
    units = _units(s_sizes)
    smr_offs, smr_ones, smr_tot = _smr_layout(hs)

    key = (tuple(s_sizes), tuple(hs), D, VARIANT)
    if key not in _cache:
        _cache[key] = _build_nc(s_sizes, hs, D, VARIANT)
    nc = _cache[key]

    in_maps = []
    scatter = []  # (core, unit_idx, idx_global_rows)
    for c in range(N_CORES):
        buf = np.zeros((PART, ftot), dtype=np.float32)
        smr = np.zeros((1, smr_tot), dtype=np.float32)
        smr[0, smr_ones:smr_ones + PART] = 1.0
        in_off = 0
        for j, s in enumerate(s_sizes):
            h = hs[j]
            a, h_off, _ = _slot_assign(c, j, hs, split, rank, A, H)
            if a >= 0:
                n_a = int(counts[a])
                idx = order[starts[a]:starts[a] + n_a]
                w1p = (W1_embs[a].reshape(kd, PART, H)
                       [:, :, h_off:h_off + h]
                       .transpose(1, 0, 2).reshape(PART, kd * h))
                if n_a > 0:
                    pidx = np.concatenate([idx, np.repeat(idx[:1], s - n_a)])
                    xtp = (X[pidx].T.reshape(kd, PART, s)
                           .transpose(1, 0, 2).reshape(PART, kd * s))
                else:
                    xtp = np.zeros((PART, kd * s), dtype=np.float32)
                for g in range(2):
                    base = in_off + g * fs[j]
                    buf[:, base:base + kh * h] = (
                        w1p[:, g * kh * h:(g + 1) * kh * h])
                    buf[:, base + offs[j]:base + offs[j] + kh * s] = (
                        xtp[:, g * kh * s:(g + 1) * kh * s])
                smr[0, smr_offs[j]:smr_offs[j] + h] = (
                    b1_embs[a][h_off:h_off + h])
                w2c = W2_embs[a].reshape(H, T)
                for t in range(T):
                    base = smr_offs[j] + h + t * (h + 2)
                    smr[0, base:base + h] = w2c[h_off:h_off + h, t]
                    smr[0, base + h] = (
                        b2_embs[a][t] if h_off == 0 else 0.0)
                for ui, (jj, s0, sc) in enumerate(units):
                    if jj == j and s0 < n_a:
                        scatter.append((c, ui, idx[s0:s0 + sc]))
            in_off += 2 * fs[j]
        if VARIANT == "bf16":
            import ml_dtypes
            buf = buf.astype(ml_dtypes.bfloat16)
            smr = smr.astype(ml_dtypes.bfloat16)
        in_maps.append({"IN": buf, "SMALLR": smr})

    if trace:
        _install_ntff_hook()
    from concourse import bass_utils
    bass_utils.upload_artifacts = lambda tmpdir: str(tmpdir)
    res = bass_utils.run_bass_kernel_spmd(
        nc, in_maps, list(range(N_CORES)), trace=trace)

    # gather: split-aspect halves produce partial logits -> accumulate
    out = np.zeros((B, T), dtype=np.float32)
    for c, ui, idx in scatter:
        out[idx] += res.results[c]["OUT"][:len(idx), T * ui:T * ui + T]
    return out, res


def kernel(**inputs):
    out, _ = _run(**inputs)
    return out


# revision 34
# speedup vs baseline: 1.1121x; 1.0016x over previous
"""Trainium2 Bass kernel for per-aspect 2-layer MLP (embedding-lookup MLP).

Reference computation (B=1024, D=768, H=256, A=20, T=2):
    W1 = W1_embs[aspect_ids].reshape(B, D, H)
    out1 = relu(X @batched W1 + b1_embs[aspect_ids])
    logits = out1 @batched W2_embs[aspect_ids].reshape(B, H, T) + b2_embs[aspect_ids]

Strategy: only A=20 distinct aspects exist, so group samples by aspect on
the host and turn the per-sample batched matvec into one dense matmul per
aspect.  Shard the aspect-groups across the 8 NeuronCores so the big
weight table is read from HBM exactly once chip-wide (~16MB total)
instead of once per sample (~800MB):

  - A // 8 = 2 "full" slot classes: core c serves aspects rank[c] and
    rank[8+c] (groups assigned by size rank so slot j has the same padded
    size S_j on every core -> SPMD-uniform program).
  - the A % 8 = 4 leftover aspects are split along the HIDDEN dim: two
    cores each take 128 of the 256 hidden units (relu is elementwise, so
    each half is self-contained through layer 1; layer 2 contributions
    are partial sums the host adds during the gather).  This balances W1
    bytes perfectly: 20 x 786KB / 8 = 1.97MB per core.

Device program per slot (S = padded group size, chunks of <=128 samples):
  - two ~0.5MB DMAs (k<3 half on the SP HWDGE ring, k>=3 half on the ACT
    ring; per-ring FIFO keeps arrivals in compute order, two rings keep
    two transfers in flight) load the host-packed [128, F] half-slabs:
    W1 as [128,h_j] rhs chunks + X^T as [128,S] lhsT (stationary) chunks.
  - layer 1 on PE: psum[S,h_j] accumulates 6 matmuls in float32r
    (single-pass fp32 mode, full 1 cycle/row stream rate at N>=256) plus
    a 7th K=1 matmul (ones[1,S] x b1[1,h_j]) adding the bias.
  - w2 columns are replicated across partitions by a K=1 PE matmul
    (ones[128] x w2row) and copied to SBUF.
  - layer 2 on DVE: one fused scalar_tensor_tensor per logit column
    computes (psum max 0) * w2col with accum_out = per-sample sum (relu
    fused in, layer-1 psum read directly), then a tiny tensor_add adds b2
    (host packs b2 = 0 for the second half of a split aspect).
  - logits live as [S,2] column pairs of a [128, 2*n_units] tile; one
    final DMA (on the otherwise-idle ACT ring) stores it.

BIR post-passes work around toolchain limits and shave startup: splitting
>1-sync-wait instructions (this walrus rejects them), and hoisting the
wait-free input DMA triggers above the program entry barrier so HBM
transfers run while engines initialize (~6us saved).

float32r is the TRN2 single-pass fp32 matmul mode: ~1.5e-4 relative
error vs ~1e-7 for the 2-pass fp32 mode, ~2.7x faster.  VARIANT="fp32"
gives bit-accurate 2-pass matmuls at ~+6us; "bf16" halves DMA bytes but
measured slower here (cold-PE-bound) with ~3.5e-3 error.
"""

import numpy as np

N_CORES = 8
PART = 128
VARIANT = "fp32r"  # "fp32r" | "fp32" | "bf16"

_cache: dict = {}


# ───────────────────────── BIR post-passes ─────────────────────────

def _split_excess_waits(nc):
    """This walrus build rejects >1 sync-wait on one instruction (seen on
    the TileContext tail Drain).  Hoist excess sem waits onto preceding
    NoOps on the same engine — semantically identical (program order)."""
    import concourse.mybir as mybir
    import bass_rust

    n_new = 0
    for f in nc.m.functions:
        for bb in f.blocks:
            insts = bb.instructions
            out = []
            changed = False
            for inst in insts:
                si = inst.sync_info
                if si is not None and si.on_wait and len(si.on_wait) > 1:
                    waits = list(si.on_wait)
                    keep = [w for w in waits if w.wait_reg is not None]
                    movable = [w for w in waits if w.wait_reg is None]
                    while len(keep) < 1 and movable:
                        keep.append(movable.pop())
                    for w in movable:
                        nop = mybir.InstNoOp(
                            name=f"waitsplit_{n_new}", engine=inst.engine,
                            sync_info=bass_rust.SyncInfo(on_wait=[w], on_update=[]))
                        n_new += 1
                        out.append(nop)
                    inst.sync_info = bass_rust.SyncInfo(
                        on_wait=keep, on_update=list(si.on_update))
                    changed = True
                out.append(inst)
            if changed:
                bb.instructions = out
    return n_new


def _hoist_initial_dmas(nc):
    """Move wait-free input-DMA triggers from the tile body to before the
    program's entry barrier on their issuing engine, so HBM transfers start
    while the engines are still initializing (saves ~6us of startup)."""
    import concourse.mybir as mybir

    f = nc.m.functions[0]
    bbs = list(f.blocks)
    if len(bbs) < 2:
        return 0
    main_bb, body_bb = bbs[0], bbs[1]

    body = body_bb.instructions
    hoisted = {}  # engine -> list[inst]
    remaining = []
    blocked = set()  # engines whose stream hit a non-hoistable inst
    for inst in body:
        eng = inst.engine
        si = inst.sync_info
        is_dma = isinstance(inst, mybir.InstDMACopy)
        waitfree = si is None or not si.on_wait
        if is_dma and waitfree and eng not in blocked:
            hoisted.setdefault(eng, []).append(inst)
        else:
            if eng != mybir.EngineType.Unassigned:
                blocked.add(eng)
            remaining.append(inst)
    if not hoisted:
        return 0

    # Within each engine, put the slow single-partition SMALLR transfer
    # AFTER the first big IN slab (Tile's scheduler orders it first, which
    # delays the whole b-half stream by ~1.7us behind a 1-partition DMA).
    def _src_name(inst):
        try:
            return inst.ins[0].bass_ap.tensor.name
        except Exception:
            return ""
    for eng, lst in hoisted.items():
        bigs = [i for i in lst if _src_name(i) == "IN"]
        smalls = [i for i in lst if _src_name(i) != "IN"]
        if bigs and smalls:
            hoisted[eng] = bigs[:1] + smalls + bigs[1:]

    main = main_bb.instructions
    out = []
    placed = set()
    # insert right before the engine's entry Drain
    for inst in main:
        if (isinstance(inst, mybir.InstDrain) and inst.engine in hoisted
                and inst.engine not in placed):
            out.extend(hoisted[inst.engine])
            placed.add(inst.engine)
        out.append(inst)
    for eng, insts in hoisted.items():
        if eng not in placed:
            out.extend(insts)
    main_bb.instructions = out
    body_bb.instructions = remaining
    return sum(len(v) for v in hoisted.values())


# ───────────────────────── shared layout ─────────────────────────

T_OUT = 2


def _classes(A, H, mh):
    """Slot classes: (n_full full-H slots, split: bool).  The leftover
    A % 8 aspects are h-split across two cores when that covers <= all
    cores and H has an even number of 128-chunks."""
    n_full = A // N_CORES
    rem = A % N_CORES
    hs = [H] * n_full
    split = False
    if rem:
        if mh % 2 == 0 and 2 * rem <= N_CORES:
            hs.append(H // 2)
            split = True
        else:
            hs.append(H)
    return hs, split


def _layout(s_sizes, hs, d):
    """Per slot TWO half-slabs (k-groups), each [128, 3*h_j + 3*S_j]."""
    kd = d // PART
    kh = kd // 2
    offs, fs = [], []
    for s, h in zip(s_sizes, hs):
        o_xt = kh * h
        f = o_xt + kh * s
        f += (-f) % 8
        offs.append(o_xt)
        fs.append(f)
    return offs, fs


def _smr_layout(hs):
    """SMALLR row: per-slot [b1[h_j] | w2col0,b2_0 [h_j+2] | w2col1,b2_1
    [h_j+2]], then ones[PART]."""
    offs = []
    off = 0
    for h in hs:
        offs.append(off)
        per = h + T_OUT * (h + 2)
        per += (-per) % 8
        off += per
    return offs, off, off + PART  # slot offsets, ones offset, total


def _units(s_sizes):
    """(slot, s0, sc) chunks of <=128 samples."""
    us = []
    for j, s in enumerate(s_sizes):
        for s0 in range(0, s, PART):
            us.append((j, s0, min(PART, s - s0)))
    return us


# ───────────────────────── device program ─────────────────────────

def _build_nc(s_sizes, hs, d, variant):
    import concourse.bass as bass
    import concourse.mybir as mybir
    from concourse.tile import TileContext

    fp32 = mybir.dt.float32
    mmdt = {"fp32r": mybir.dt.float32r,
            "bf16": mybir.dt.bfloat16,
            "fp32": fp32}[variant]
    kd = d // PART
    kh = kd // 2
    T = T_OUT
    offs, fs = _layout(s_sizes, hs, d)
    ftot = 2 * sum(fs)
    units = _units(s_sizes)
    n_slots = len(s_sizes)
    smr_offs, smr_ones, smr_tot = _smr_layout(hs)
    hmax = max(hs)

    nc = bass.Bass()
    IN = nc.dram_tensor("IN", [PART, ftot], mmdt, kind="ExternalInput")
    SMALLR = nc.dram_tensor("SMALLR", [1, smr_tot], mmdt, kind="ExternalInput")
    OUT = nc.dram_tensor("OUT", [PART, T * len(units)], fp32,
                         kind="ExternalOutput")

    with TileContext(nc) as tc:
        with tc.tile_pool(name="inp", bufs=2 * n_slots) as inp_pool, \
             tc.tile_pool(name="smallp", bufs=1) as small_pool, \
             tc.tile_pool(name="w2sbp", bufs=T * n_slots) as w2sb_pool, \
             tc.tile_pool(name="scrp", bufs=2) as scr_pool, \
             tc.tile_pool(name="outp", bufs=1) as out_pool, \
             tc.tile_pool(name="ps1", bufs=2, space="PSUM") as ps1_pool, \
             tc.tile_pool(name="psw", bufs=T * n_slots, space="PSUM") as psw_pool:

            out_sb = out_pool.tile([PART, T * len(units)], fp32)
            small_t = small_pool.tile([1, smr_tot], mmdt)

            # prefetch all input half-slabs, k<3 halves on the sync ring and
            # k>=3 halves on the scalar ring (per-ring FIFO => data lands in
            # compute order; two rings keep two transfers in flight).
            in_ts = []
            in_off = 0
            for j in range(n_slots):
                pair = []
                for g in range(2):
                    in_t = inp_pool.tile([PART, fs[j]], mmdt, tag="in_t")
                    eng = nc.sync if g == 0 else nc.scalar
                    eng.dma_start(
                        out=in_t[:], in_=IN[:, in_off:in_off + fs[j]])
                    pair.append(in_t)
                    in_off += fs[j]
                    if j == 0 and g == 1:
                        nc.scalar.dma_start(out=small_t[:], in_=SMALLR[:])
                in_ts.append(pair)

            ones_full = small_t[0:1, smr_ones:smr_ones + PART]

            # replicate each slot's w2 column (+b2) across all partitions on
            # the PE: psum[p, f] = ones[p] * w2row[f]; copy to SBUF for DVE.
            w2sb = []
            for j in range(n_slots):
                h = hs[j]
                for t in range(T):
                    wp = psw_pool.tile([PART, h + 2], fp32, tag="w2ps")
                    src = small_t[0:1, smr_offs[j] + h + t * (h + 2):
                                  smr_offs[j] + h + (t + 1) * (h + 2)]
                    nc.tensor.matmul(wp[:], ones_full, src,
                                     start=True, stop=True)
                    wsb = w2sb_pool.tile([PART, hmax + 2], fp32, tag="w2sb")
                    nc.vector.tensor_copy(out=wsb[:, 0:h + 2], in_=wp[:])
                    w2sb.append(wsb)

            for ui, (j, s0, sc) in enumerate(units):
                s = s_sizes[j]
                h = hs[j]
                o_xt = offs[j]
                b1row = small_t[0:1, smr_offs[j]:smr_offs[j] + h]
                ones = small_t[0:1, smr_ones + 0:smr_ones + sc]

                ps = ps1_pool.tile([sc, h], fp32, tag="ps")
                for k in range(kd):
                    in_t = in_ts[j][k // kh]
                    kk = k % kh
                    nc.tensor.matmul(
                        ps[:],
                        in_t[:, o_xt + kk * s + s0:o_xt + kk * s + s0 + sc],
                        in_t[:, kk * h:(kk + 1) * h],
                        start=(k == 0), stop=False)
                nc.tensor.matmul(
                    ps[:], ones, b1row, start=False, stop=True)

                for t in range(T):
                    wsb = w2sb[j * T + t]
                    scr = scr_pool.tile([PART, hmax], fp32, tag=f"scr{t}")
                    acc = scr_pool.tile([PART, 1], fp32, tag=f"acc{t}")
                    # logits col = sum_h(relu(psum) * w2[:,t]) in one DVE op
                    nc.vector.scalar_tensor_tensor(
                        out=scr[:sc, 0:h], in0=ps[:], scalar=0.0,
                        in1=wsb[:sc, 0:h],
                        op0=mybir.AluOpType.max,
                        op1=mybir.AluOpType.mult,
                        accum_out=acc[:sc, 0:1])
                    nc.vector.tensor_add(
                        out=out_sb[:sc, T * ui + t:T * ui + t + 1],
                        in0=acc[:sc, 0:1],
                        in1=wsb[:sc, h:h + 1])
            nc.scalar.dma_start(out=OUT[:], in_=out_sb[:])

    _split_excess_waits(nc)
    _hoist_initial_dmas(nc)
    return nc


# ───────────────────────── host side ─────────────────────────

def _install_ntff_hook():
    import sys, types
    if "antenv.axon_hooks" in sys.modules:
        return
    import antenv
    from trn_agent_boot.trn_boot import _ntff_profile_via_ctypes
    mod = types.ModuleType("antenv.axon_hooks")
    hook = _ntff_profile_via_ctypes('/opt/axon/libaxon_pjrt.so')
    mod.get_axon_ntff_profile_hook = lambda: hook
    mod.set_axon_ntff_profile_hook = lambda h: None
    sys.modules["antenv.axon_hooks"] = mod
    antenv.axon_hooks = mod


def _slot_assign(c, j, hs, split, rank, A, H):
    """Aspect + hidden-range served by (core c, slot j).
    Returns (aspect or -1, h_off, h_len)."""
    h = hs[j]
    if h == H or not split or j < len(hs) - 1:
        r = j * N_CORES + c
        return (int(rank[r]) if r < A else -1), 0, h
    # split class: two cores per aspect, one H-half each
    base = j * N_CORES
    ai = c // 2
    r = base + ai
    if r < A:
        return int(rank[r]), (c % 2) * h, h
    return -1, 0, h


def _run(X, aspect_ids, W1_embs, b1_embs, W2_embs, b2_embs, trace=False):
    B, D = X.shape
    A, H = b1_embs.shape
    T = b2_embs.shape[1]
    assert D % PART == 0 and H % PART == 0 and T == T_OUT
    kd, mh = D // PART, H // PART
    kh = kd // 2

    X = np.ascontiguousarray(X, dtype=np.float32)
    W1_embs = np.ascontiguousarray(W1_embs, dtype=np.float32)
    b1_embs = np.ascontiguousarray(b1_embs, dtype=np.float32)
    W2_embs = np.ascontiguousarray(W2_embs, dtype=np.float32)
    b2_embs = np.ascontiguousarray(b2_embs, dtype=np.float32)
    ids = np.asarray(aspect_ids).astype(np.int64)

    order = np.argsort(ids, kind="stable")
    counts = np.bincount(ids, minlength=A)
    starts = np.concatenate([[0], np.cumsum(counts)])
    rank = np.argsort(-counts, kind="stable")

    hs, split = _classes(A, H, mh)
    n_slots = len(hs)
    s_sizes = []
    for j in range(n_slots):
        if split and j == n_slots - 1:
            cls = rank[j * N_CORES:A]
        else:
            cls = rank[j * N_CORES:(j + 1) * N_CORES]
        smax = max(1, int(counts[cls].max()) if len(cls) else 1)
        smax += (-smax) % 8
        s_sizes.append(smax)

    offs, fs = _layout(s_sizes, hs, D)
    ftot = 2 * sum(fs)
    units = _units(s_sizes)
    smr_offs, smr_ones, smr_tot = _smr_layout(hs)

    key = (tuple(s_sizes), tuple(hs), D, VARIANT)
    if key not in _cache:
        _cache[key] = _build_nc(s_sizes, hs, D, VARIANT)
    nc = _cache[key]

    in_maps = []
    scatter = []  # (core, unit_idx, idx_global_rows)
    for c in range(N_CORES):
        buf = np.zeros((PART, ftot), dtype=np.float32)
        smr = np.zeros((1, smr_tot), dtype=np.float32)
        smr[0, smr_ones:smr_ones + PART] = 1.0
        in_off = 0
        for j, s in enumerate(s_sizes):
            h = hs[j]
            a, h_off, _ = _slot_assign(c, j, hs, split, rank, A, H)
            if a >= 0:
                n_a = int(counts[a])
                idx = order[starts[a]:starts[a] + n_a]
                w1p = (W1_embs[a].reshape(kd, PART, H)
                       [:, :, h_off:h_off + h]
                       .transpose(1, 0, 2).reshape(PART, kd * h))
                if n_a > 0:
                    pidx = np.concatenate([idx, np.repeat(idx[:1], s - n_a)])
                    xtp = (X[pidx].T.reshape(kd, PART, s)
                           .transpose(1, 0, 2).reshape(PART, kd * s))
                else:
                    xtp = np.zeros((PART, kd * s), dtype=np.float32)
                for g in range(2):
                    base = in_off + g * fs[j]
                    buf[:, base:base + kh * h] = (
                        w1p[:, g * kh * h:(g + 1) * kh * h])
                    buf[:, base + offs[j]:base + offs[j] + kh * s] = (
                        xtp[:, g * kh * s:(g + 1) * kh * s])
                smr[0, smr_offs[j]:smr_offs[j] + h] = (
                    b1_embs[a][h_off:h_off + h])
                w2c = W2_embs[a].reshape(H, T)
                for t in range(T):
                    base = smr_offs[j] + h + t * (h + 2)
                    smr[0, base:base + h] = w2c[h_off:h_off + h, t]
                    smr[0, base + h] = (
                        b2_embs[a][t] if h_off == 0 else 0.0)
                for ui, (jj, s0, sc) in enumerate(units):
                    if jj == j and s0 < n_a:
                        scatter.append((c, ui, idx[s0:s0 + sc]))
            in_off += 2 * fs[j]
        if VARIANT == "bf16":
            import ml_dtypes
            buf = buf.astype(ml_dtypes.bfloat16)
            smr = smr.astype(ml_dtypes.bfloat16)
        in_maps.append({"IN": buf, "SMALLR": smr})

    if trace:
        _install_ntff_hook()
    from concourse import bass_utils
    bass_utils.upload_artifacts = lambda tmpdir: str(tmpdir)
    res = bass_utils.run_bass_kernel_spmd(
        nc, in_maps, list(range(N_CORES)), trace=trace)

    # gather: split-aspect halves produce partial logits -> accumulate
    out = np.zeros((B, T), dtype=np.float32)
    for c, ui, idx in scatter:
        out[idx] += res.results[c]["OUT"][:len(idx), T * ui:T * ui + T]
    return out, res


def kernel(**inputs):
    out, _ = _run(**inputs)
    return out
